# revision 8
# baseline (speedup 1.0000x reference)
"""ArcFace logits kernel for 8 trn2 NeuronCores (class-axis model parallel).

kernel(input, weight, label) -> [1024, 100000] f32 scaled-margin logits.

Strategy (v2): the O(N*C*D) cosine GEMM runs on device; all O((N+C)*D)
layout/precision prep runs on host so the device program is a pure
matmul + evict + store pipeline near its rooflines:

Host prep (per core shard of 12500 classes):
  - xn = l2norm(x) f64, laid out as xnT [4 kt, 128 d, 1024 n] and cast to
    fp8e4 (TRN e4m3, max 240) -- single rounding, matches ml_dtypes.
  - wn = l2norm(w_shard) f32, laid out wnT [4 kt, 128 d, 12800 c-padded]
    cast fp8e4.  Logical contraction index d = kt*128 + partition.
Device (SPMD, 8 cores):
  - xnT resident in SBUF (4 KB/partition), wnT resident (51 KB/partition,
    loaded in 5 column groups so MMs start after the first group lands).
  - 25 chunks x 8 batch-tiles: 2 DoubleRow fp8 matmuls (contraction 256
    each) accumulate [128,512] f32 in PSUM; evict with x30 scale to fp16
    (ACT/DVE alternating); DMA the [128, <=512] fp16 slab to DRAM out.
Host assemble:
  - concat 8 fp16 slabs -> f32 [1024, 100000]
  - margin positions: cos_t recomputed exactly (f64) from xn/wn rows at
    the 1024 labels, phi overwrites out[rows, label] (same math as ref).

Numerics (validated against the fixed seed-0 data in f64 simulation):
  fp8 x/w + fp16 out -> rel err 0.0165 vs gate 0.02; bf16 mode 0.001.
MODE="bf16" is the fallback (4 plain MMs instead of 2 DoubleRow MMs,
bf16 operands; same structure, ~1.6x slower PE).
"""

import math
from contextlib import ExitStack

import numpy as np
import ml_dtypes

import concourse.bass as bass
import concourse.bacc as bacc
import concourse.mybir as mybir
from concourse.tile import TileContext
from concourse.bass_utils import run_bass_kernel_spmd

F32 = mybir.dt.float32
F16 = mybir.dt.float16
BF16 = mybir.dt.bfloat16
FP8 = mybir.dt.float8e4

N = 1024          # batch
D = 512           # in_features
C_TOTAL = 100000  # out_features
N_CORES = 8
C_PER = C_TOTAL // N_CORES     # 12500 real classes per core
C_PAD = 12800                  # 25 chunks of 512
KT = D // 128                  # 4 k-subtiles
NT = N // 128                  # 8 batch tiles
N_CHUNKS = C_PAD // 512        # 25
GROUPS = 5                     # W streamed in 5 column groups
GW = C_PAD // GROUPS           # 2560 cols per group

SCALE = 30.0
MARGIN = 0.5
COS_M = math.cos(MARGIN)
SIN_M = math.sin(MARGIN)
TH = math.cos(math.pi - MARGIN)
MM = math.sin(math.pi - MARGIN) * MARGIN

MODE = "fp8"   # "fp8" (DoubleRow) | "bf16" (fallback)
PRESCALE = 32.0  # fp8 mode: x,w scaled by 32 before quantization (power of
                 # 2, exact), compensated in the evict scale 30/1024.


def build_nc(mode=MODE):
    in_dt = FP8 if mode == "fp8" else BF16
    ev_scale = SCALE / (PRESCALE * PRESCALE) if mode == "fp8" else SCALE
    nc = bacc.Bacc(None, target_bir_lowering=False, debug=False)
    xt = nc.declare_dram_parameter("xt", [KT, 128, N], in_dt, isOutput=False)
    wt = nc.declare_dram_parameter("wt", [KT, 128, C_PAD], in_dt, isOutput=False)
    out = nc.declare_dram_parameter("out", [N, C_PER], F16, isOutput=True)

    # quads: groups of chunks -> one wide out-write per (quad, b).  Small
    # first quad (early MM start during lead-in) and small last quads
    # (fast drain); the final 512-chunk computes only 256 cols (212 real).
    quads = [(0, 2), (2, 4), (6, 4), (10, 4), (14, 4), (18, 4), (22, 2), (24, 1)]

    with ExitStack() as ctx:
        tc = ctx.enter_context(TileContext(nc))

        xpool = ctx.enter_context(tc.tile_pool(name="xnt", bufs=1))
        wpool = ctx.enter_context(tc.tile_pool(name="wtp", bufs=1))
        opool = ctx.enter_context(tc.tile_pool(name="opool", bufs=6))
        psum = ctx.enter_context(tc.tile_pool(name="psum", space="PSUM", bufs=2))

        xnt = xpool.tile([128, KT, N], in_dt, name="xnt")
        for k in range(KT):
            nc.sync.dma_start(out=xnt[:, k, :], in_=xt[k])

        wtiles = []
        for qi, (g0, qw) in enumerate(quads):
            t = wpool.tile([128, KT, qw * 512], in_dt, tag=f"w{qi}", name=f"w{qi}")
            w_eng = nc.sync if qi == 0 else nc.gpsimd
            for k in range(KT):
                w_eng.dma_start(
                    out=t[:, k, :], in_=wt[k, :, g0 * 512:(g0 + qw) * 512]
                )
            wtiles.append(t)

        def evict(gi, b, dst, src):
            if (b + gi) % 2 == 0:
                nc.scalar.activation(
                    out=dst, in_=src,
                    func=mybir.ActivationFunctionType.Copy, scale=ev_scale,
                )
            else:
                nc.vector.tensor_scalar_mul(dst, src, ev_scale)

        for qi, (g0, qw) in enumerate(quads):
            for b in range(NT):
                bs = slice(b * 128, (b + 1) * 128)
                # per-chunk compute width: trim the final chunk to 256 cols
                cws = [
                    256 if g0 + gi == N_CHUNKS - 1 else 512 for gi in range(qw)
                ]
                pts = [
                    psum.tile([128, 512], F32, tag="opsum", name=f"pt{gi}", bufs=8)
                    for gi in range(qw)
                ]
                ost = opool.tile([128, qw * 512], F16, name="ost", tag=f"ost{qw}")
                if mode == "fp8":
                    # j outer so consecutive MMs share the stationary lhsT;
                    # evicts interleave with the j=1 stream to free banks early
                    for gi in range(qw):
                        nc.tensor.matmul(
                            pts[gi][:, 0:cws[gi]],
                            lhsT=xnt[:, 0:2, bs],
                            rhs=wtiles[qi][:, 0:2, gi * 512:gi * 512 + cws[gi]],
                            start=True, stop=False,
                            perf_mode=mybir.MatmulPerfMode.DoubleRow,
                        )
                    for gi in range(qw):
                        nc.tensor.matmul(
                            pts[gi][:, 0:cws[gi]],
                            lhsT=xnt[:, 2:4, bs],
                            rhs=wtiles[qi][:, 2:4, gi * 512:gi * 512 + cws[gi]],
                            start=False, stop=True,
                            perf_mode=mybir.MatmulPerfMode.DoubleRow,
                        )
                        evict(gi, b, ost[:, gi * 512:gi * 512 + cws[gi]],
                              pts[gi][:, 0:cws[gi]])
                else:
                    for k in range(KT):
                        for gi in range(qw):
                            nc.tensor.matmul(
                                pts[gi][:, 0:cws[gi]],
                                lhsT=xnt[:, k, bs],
                                rhs=wtiles[qi][:, k, gi * 512:gi * 512 + cws[gi]],
                                start=(k == 0), stop=(k == KT - 1),
                            )
                    for gi in range(qw):
                        evict(gi, b, ost[:, gi * 512:gi * 512 + cws[gi]],
                              pts[gi][:, 0:cws[gi]])
                ow = min(qw * 512, C_PER - g0 * 512)   # fp16 cols actually written
                nc.sync.dma_start(
                    out=out[bs, g0 * 512:g0 * 512 + ow],
                    in_=ost[:, 0:ow],
                )
    nc.compile()
    return nc


_NC_CACHE = {}
_HOST_CTX = {}


def _get_nc():
    if "nc" not in _NC_CACHE:
        _NC_CACHE["nc"] = build_nc()
    return _NC_CACHE["nc"]


def prep_in_maps(input, weight, mode=MODE):
    np_in = ml_dtypes.float8_e4m3 if mode == "fp8" else ml_dtypes.bfloat16
    x64 = np.asarray(input, dtype=np.float64)
    xn = x64 / np.maximum(np.linalg.norm(x64, axis=1, keepdims=True), 1e-12)
    _HOST_CTX["xn"] = xn
    _HOST_CTX["weight"] = weight
    ps = PRESCALE if mode == "fp8" else 1.0
    xt = np.ascontiguousarray(xn.T * ps).reshape(KT, 128, N).astype(np_in)

    w = np.asarray(weight, dtype=np.float32)
    in_maps = []
    for i in range(N_CORES):
        wi = w[i * C_PER:(i + 1) * C_PER]
        nrm = np.sqrt(np.einsum("cd,cd->c", wi, wi, dtype=np.float64))
        wn = wi / np.maximum(nrm, 1e-12).astype(np.float32)[:, None]
        wti = np.zeros((D, C_PAD), dtype=np.float32)
        wti[:, :C_PER] = wn.T * np.float32(ps)
        in_maps.append({"xt": xt, "wt": wti.reshape(KT, 128, C_PAD).astype(np_in)})
    return in_maps


def assemble(results, label):
    out = np.empty((N, C_TOTAL), dtype=np.float32)
    for i in range(N_CORES):
        out[:, i * C_PER:(i + 1) * C_PER] = results[i]["out"].astype(np.float32)
    lab = np.asarray(label).astype(np.int64)
    rows = np.arange(N)
    # exact margin: recompute the 1024 true-class cosines on host in f64
    xn = _HOST_CTX["xn"]
    wrows = np.asarray(_HOST_CTX["weight"], dtype=np.float32)[lab].astype(np.float64)
    wrows /= np.maximum(np.linalg.norm(wrows, axis=1, keepdims=True), 1e-12)
    cos_t = np.einsum("nd,nd->n", xn, wrows)
    sin_t = np.sqrt(np.maximum(1.0 - cos_t * cos_t, 0.0))
    phi = cos_t * COS_M - sin_t * SIN_M
    phi = np.where(cos_t > TH, phi, cos_t - MM)
    out[rows, lab] = (SCALE * phi).astype(np.float32)
    return out


def kernel(input, weight, label):
    nc = _get_nc()
    in_maps = prep_in_maps(input, weight)
    res = run_bass_kernel_spmd(nc, in_maps, list(range(N_CORES)))
    return assemble(res.results, label)


# revision 10
# speedup vs baseline: 1.0785x; 1.0785x over previous
"""ArcFace logits kernel for 8 trn2 NeuronCores (class-axis model parallel).

kernel(input, weight, label) -> [1024, 100000] f32 scaled-margin logits.

Strategy (v2): the O(N*C*D) cosine GEMM runs on device; all O((N+C)*D)
layout/precision prep runs on host so the device program is a pure
matmul + evict + store pipeline near its rooflines:

Host prep (per core shard of 12500 classes):
  - xn = l2norm(x) f64, laid out as xnT [4 kt, 128 d, 1024 n] and cast to
    fp8e4 (TRN e4m3, max 240) -- single rounding, matches ml_dtypes.
  - wn = l2norm(w_shard) f32, laid out wnT [4 kt, 128 d, 12800 c-padded]
    cast fp8e4.  Logical contraction index d = kt*128 + partition.
Device (SPMD, 8 cores):
  - xnT resident in SBUF (4 KB/partition), wnT resident (51 KB/partition,
    loaded in 5 column groups so MMs start after the first group lands).
  - 25 chunks x 8 batch-tiles: 2 DoubleRow fp8 matmuls (contraction 256
    each) accumulate [128,512] f32 in PSUM; evict with x30 scale to fp16
    (ACT/DVE alternating); DMA the [128, <=512] fp16 slab to DRAM out.
Host assemble:
  - concat 8 fp16 slabs -> f32 [1024, 100000]
  - margin positions: cos_t recomputed exactly (f64) from xn/wn rows at
    the 1024 labels, phi overwrites out[rows, label] (same math as ref).

Numerics (validated against the fixed seed-0 data in f64 simulation):
  fp8 x/w + fp16 out -> rel err 0.0165 vs gate 0.02; bf16 mode 0.001.
MODE="bf16" is the fallback (4 plain MMs instead of 2 DoubleRow MMs,
bf16 operands; same structure, ~1.6x slower PE).
"""

import math
from contextlib import ExitStack

import numpy as np
import ml_dtypes

import concourse.bass as bass
import concourse.bacc as bacc
import concourse.mybir as mybir
from concourse.tile import TileContext
from concourse.bass_utils import run_bass_kernel_spmd

F32 = mybir.dt.float32
F16 = mybir.dt.float16
BF16 = mybir.dt.bfloat16
FP8 = mybir.dt.float8e4

N = 1024          # batch
D = 512           # in_features
C_TOTAL = 100000  # out_features
N_CORES = 8
C_PER = C_TOTAL // N_CORES     # 12500 real classes per core
C_PAD = 12800                  # 25 chunks of 512
KT = D // 128                  # 4 k-subtiles
NT = N // 128                  # 8 batch tiles
N_CHUNKS = C_PAD // 512        # 25
GROUPS = 5                     # W streamed in 5 column groups
GW = C_PAD // GROUPS           # 2560 cols per group

SCALE = 30.0
MARGIN = 0.5
COS_M = math.cos(MARGIN)
SIN_M = math.sin(MARGIN)
TH = math.cos(math.pi - MARGIN)
MM = math.sin(math.pi - MARGIN) * MARGIN

MODE = "fp8"   # "fp8" (DoubleRow) | "bf16" (fallback)
PRESCALE = 32.0  # fp8 mode: x,w scaled by 32 before quantization (power of
                 # 2, exact), compensated in the evict scale 30/1024.


def build_nc(mode=MODE):
    in_dt = FP8 if mode == "fp8" else BF16
    ev_scale = SCALE / (PRESCALE * PRESCALE) if mode == "fp8" else SCALE
    nc = bacc.Bacc(None, target_bir_lowering=False, debug=False)
    xt = nc.declare_dram_parameter("xt", [KT, 128, N], in_dt, isOutput=False)
    wt = nc.declare_dram_parameter("wt", [KT, 128, C_PAD], in_dt, isOutput=False)
    out = nc.declare_dram_parameter("out", [N, C_PER], F16, isOutput=True)

    # quads: groups of chunks -> one wide out-write per (quad, b).  Small
    # first quad (early MM start during lead-in) and small last quads
    # (fast drain); the final 512-chunk computes only 256 cols (212 real).
    quads = [(0, 2), (2, 4), (6, 4), (10, 4), (14, 4), (18, 4), (22, 2), (24, 1)]

    with ExitStack() as ctx:
        tc = ctx.enter_context(TileContext(nc))

        xpool = ctx.enter_context(tc.tile_pool(name="xnt", bufs=1))
        wpool = ctx.enter_context(tc.tile_pool(name="wtp", bufs=1))
        opool = ctx.enter_context(tc.tile_pool(name="opool", bufs=6))
        psum = ctx.enter_context(tc.tile_pool(name="psum", space="PSUM", bufs=2))

        # x and W tiles split into j-halves (k-subtiles 0-1 / 2-3) so the
        # first j=0 matmuls wait only on their own half's DMAs.
        xnt = []
        for j in range(2):
            t = xpool.tile([128, 2, N], in_dt, tag=f"x{j}", name=f"xnt{j}")
            for k in range(2):
                nc.sync.dma_start(out=t[:, k, :], in_=xt[2 * j + k])
            xnt.append(t)

        wtiles = []
        for qi, (g0, qw) in enumerate(quads):
            pair = []
            for j in range(2):
                t = wpool.tile(
                    [128, 2, qw * 512], in_dt, tag=f"w{qi}_{j}", name=f"w{qi}_{j}"
                )
                for k in range(2):
                    nc.gpsimd.dma_start(
                        out=t[:, k, :],
                        in_=wt[2 * j + k, :, g0 * 512:(g0 + qw) * 512],
                    )
                pair.append(t)
            wtiles.append(pair)

        def evict(gi, b, dst, src):
            if (b + gi) % 2 == 0:
                nc.scalar.activation(
                    out=dst, in_=src,
                    func=mybir.ActivationFunctionType.Copy, scale=ev_scale,
                )
            else:
                nc.vector.tensor_scalar_mul(dst, src, ev_scale)

        for qi, (g0, qw) in enumerate(quads):
            for b in range(NT):
                bs = slice(b * 128, (b + 1) * 128)
                # per-chunk compute width: trim the final chunk to 256 cols
                cws = [
                    256 if g0 + gi == N_CHUNKS - 1 else 512 for gi in range(qw)
                ]
                pts = [
                    psum.tile([128, 512], F32, tag="opsum", name=f"pt{gi}", bufs=8)
                    for gi in range(qw)
                ]
                ost = opool.tile([128, qw * 512], F16, name="ost", tag=f"ost{qw}")
                if mode == "fp8":
                    # j outer so consecutive MMs share the stationary lhsT;
                    # evicts interleave with the j=1 stream to free banks early
                    for gi in range(qw):
                        nc.tensor.matmul(
                            pts[gi][:, 0:cws[gi]],
                            lhsT=xnt[0][:, :, bs],
                            rhs=wtiles[qi][0][:, :, gi * 512:gi * 512 + cws[gi]],
                            start=True, stop=False,
                            perf_mode=mybir.MatmulPerfMode.DoubleRow,
                        )
                    for gi in range(qw):
                        nc.tensor.matmul(
                            pts[gi][:, 0:cws[gi]],
                            lhsT=xnt[1][:, :, bs],
                            rhs=wtiles[qi][1][:, :, gi * 512:gi * 512 + cws[gi]],
                            start=False, stop=True,
                            perf_mode=mybir.MatmulPerfMode.DoubleRow,
                        )
                        evict(gi, b, ost[:, gi * 512:gi * 512 + cws[gi]],
                              pts[gi][:, 0:cws[gi]])
                else:
                    for k in range(KT):
                        for gi in range(qw):
                            nc.tensor.matmul(
                                pts[gi][:, 0:cws[gi]],
                                lhsT=xnt[k // 2][:, k % 2, bs],
                                rhs=wtiles[qi][k // 2][:, k % 2, gi * 512:gi * 512 + cws[gi]],
                                start=(k == 0), stop=(k == KT - 1),
                            )
                    for gi in range(qw):
                        evict(gi, b, ost[:, gi * 512:gi * 512 + cws[gi]],
                              pts[gi][:, 0:cws[gi]])
                ow = min(qw * 512, C_PER - g0 * 512)   # fp16 cols actually written
                nc.sync.dma_start(
                    out=out[bs, g0 * 512:g0 * 512 + ow],
                    in_=ost[:, 0:ow],
                )
    nc.compile()
    return nc


_NC_CACHE = {}
_HOST_CTX = {}


def _get_nc():
    if "nc" not in _NC_CACHE:
        _NC_CACHE["nc"] = build_nc()
    return _NC_CACHE["nc"]


def prep_in_maps(input, weight, mode=MODE):
    np_in = ml_dtypes.float8_e4m3 if mode == "fp8" else ml_dtypes.bfloat16
    x64 = np.asarray(input, dtype=np.float64)
    xn = x64 / np.maximum(np.linalg.norm(x64, axis=1, keepdims=True), 1e-12)
    _HOST_CTX["xn"] = xn
    _HOST_CTX["weight"] = weight
    ps = PRESCALE if mode == "fp8" else 1.0
    xt = np.ascontiguousarray(xn.T * ps).reshape(KT, 128, N).astype(np_in)

    w = np.asarray(weight, dtype=np.float32)
    in_maps = []
    for i in range(N_CORES):
        wi = w[i * C_PER:(i + 1) * C_PER]
        nrm = np.sqrt(np.einsum("cd,cd->c", wi, wi, dtype=np.float64))
        wn = wi / np.maximum(nrm, 1e-12).astype(np.float32)[:, None]
        wti = np.zeros((D, C_PAD), dtype=np.float32)
        wti[:, :C_PER] = wn.T * np.float32(ps)
        in_maps.append({"xt": xt, "wt": wti.reshape(KT, 128, C_PAD).astype(np_in)})
    return in_maps


def assemble(results, label):
    out = np.empty((N, C_TOTAL), dtype=np.float32)
    for i in range(N_CORES):
        out[:, i * C_PER:(i + 1) * C_PER] = results[i]["out"].astype(np.float32)
    lab = np.asarray(label).astype(np.int64)
    rows = np.arange(N)
    # exact margin: recompute the 1024 true-class cosines on host in f64
    xn = _HOST_CTX["xn"]
    wrows = np.asarray(_HOST_CTX["weight"], dtype=np.float32)[lab].astype(np.float64)
    wrows /= np.maximum(np.linalg.norm(wrows, axis=1, keepdims=True), 1e-12)
    cos_t = np.einsum("nd,nd->n", xn, wrows)
    sin_t = np.sqrt(np.maximum(1.0 - cos_t * cos_t, 0.0))
    phi = cos_t * COS_M - sin_t * SIN_M
    phi = np.where(cos_t > TH, phi, cos_t - MM)
    out[rows, lab] = (SCALE * phi).astype(np.float32)
    return out


def kernel(input, weight, label):
    nc = _get_nc()
    in_maps = prep_in_maps(input, weight)
    res = run_bass_kernel_spmd(nc, in_maps, list(range(N_CORES)))
    return assemble(res.results, label)
